# revision 15
# baseline (speedup 1.0000x reference)
"""Trainium2 Bass kernel for nn_Decoder (Hawkes intensity decoder).

Contract: kernel(**inputs) takes FULL unsharded inputs (as produced by the
reference's setup_inputs) and returns the full (lambda_src, lambda_dst,
return_time_pred) tuple.

Sharding (8 NeuronCores):
  - lambda_src/lambda_dst (B=512 x N=50000): node-sharded. Core c computes
    ALL 512 events against its 6250-node slice of all_embeddings. This cuts
    the per-core input DMA 8x vs batch-sharding (only the node slice is
    read) while output DMA (the roofline: 2 x 102.4MB fp32) is identical.
  - return_time_pred (B=512): batch-sharded, 64 events per core.

Per-core math. hawkes_intensity separates per (event b, node n):
    g[b,n] = z_ev[b].Wa[et_b] + emb[n].Wb[et_b] + bias[et_b]
             + alpha[et_b]*exp(-w_t[et_b]*td_b/100)
so with x = g/(psi+1e-7),
    lambda[b,n] = psi_e*(logaddexp(0,-x)+x) = psi_e*ln(1+exp(x)).
The node-independent part folds into a per-event scalar w_b; the node part
is a K=32 matmul row: x[b,n] = CW[b,:].emb[n,:] + w_b with
CW[b,:] = Wb[et_b]/(psi_e+1e-7). On device:
    PE   : PSUM[128ev, n] = cwT^T @ embT     (fp32r, K=32)
    ACT  : e = Exp(PSUM + w_b)  (per-partition bias)
    ACT  : l = Ln(e + 1)
    DVE  : out = l * psi_b      (per-partition scalar)
    DMA  : out -> lam[...]
|x| <~ 25 for any plausible input here, so Ln(1+Exp(x)) is overflow-safe in
fp32 and equals the reference's stable logaddexp form.

return_time_pred per core (64 events):
    E[b,s] = Exp(s * (-w_t_b*1e-4))         (s = 0..1000 row from host)
    F = Exp(rt_scale_b*E + rt_bias_b); I = psi_b*Ln(F+1)      # intensity
    transpose I -> (s,b); cumsum over s via triangular/ones matmuls
    X = Exp(-0.01*cum); ts = I_T*X*td_s; rtp = 0.01*sum(ts) - 0.005*ts[1000]
"""

import numpy as np

N_NODES = 50000
B = 512
D = 32
NCORES = 8
NC_NODES = N_NODES // NCORES  # 6250
NC_EV = B // NCORES  # 64
S = 1001
S_BLKS = 8  # ceil(1001/128); last block has 105 rows
LAST_BLK_ROWS = S - 7 * 128  # 105
TRAIN_TD_MAX = 100.0
TIMESTEP = 0.01

_PROGRAM_CACHE = {}


def _build_program():
    """Build + compile the SPMD Bass program (identical on all 8 cores)."""
    import concourse.bass as bass
    import concourse.mybir as mybir
    from concourse import bacc, tile

    dt = mybir.dt
    AF = mybir.ActivationFunctionType

    # Both Exp and Ln live in the 'natural_log_exp_and_others' activation
    # table set; left to itself the table-load pass picks per-function sets
    # and the Scalar engine reloads tables on every Exp<->Ln alternation
    # (~1.3us each, ~46us total here). Restrict selection to the shared set
    # (other sets are emptied, keeping dict order so act_func_set_id indices
    # stay aligned with act_info.json).
    from concourse.hw_specs import get_activation_tables as _real_gat

    def _patched_gat(arch):
        tabs = _real_gat(arch)
        return {
            k: (v if k == "natural_log_exp_and_others" else set())
            for k, v in tabs.items()
        }

    bacc.get_activation_tables = _patched_gat

    nc = bacc.Bacc(
        "TRN2",
        target_bir_lowering=False,
        debug=False,
        num_devices=NCORES,
    )

    # ---- DRAM parameters -------------------------------------------------
    embT_d = nc.declare_dram_parameter("embT", [D, NC_NODES], dt.float32, isOutput=False)
    cwT_d = nc.declare_dram_parameter("cwT", [D, 1024], dt.float32, isOutput=False)
    par_d = nc.declare_dram_parameter("par", [128, 12], dt.float32, isOutput=False)
    rtpar_d = nc.declare_dram_parameter("rtpar", [NC_EV, 4], dt.float32, isOutput=False)
    consts_d = nc.declare_dram_parameter("consts", [128, 384], dt.float32, isOutput=False)
    # tdc cols 0..7: td value 0.01*s per (partition, block); cols 8..15:
    # trapezoid weights (0.01 for s<1000, 0.005 at s=1000, 0 beyond)
    tdc_d = nc.declare_dram_parameter("tdc", [128, 2 * S_BLKS], dt.float32, isOutput=False)
    tdb_d = nc.declare_dram_parameter("tdb", [NC_EV, S], dt.float32, isOutput=False)

    lam_d = nc.declare_dram_parameter("lam", [1024, NC_NODES], dt.float32, isOutput=True)
    rtp_d = nc.declare_dram_parameter("rtp", [1, NC_EV], dt.float32, isOutput=True)

    F32R = dt.float32r

    with tile.TileContext(nc) as tc:
        with tc.tile_pool(name="const", bufs=1) as cpool:
            sb_embT = cpool.tile([D, NC_NODES], dt.float32)
            sb_cwT = cpool.tile([D, 1024], dt.float32)
            sb_embTr = cpool.tile([D, NC_NODES], F32R)
            sb_cwTr = cpool.tile([D, 1024], F32R)
            sb_par = cpool.tile([128, 12], dt.float32)
            sb_rtpar = cpool.tile([NC_EV, 4], dt.float32)
            sb_consts = cpool.tile([128, 384], dt.float32)
            sb_tdc = cpool.tile([128, 2 * S_BLKS], dt.float32)
            sb_tdb = cpool.tile([NC_EV, S], dt.float32)

            # Split the big embT load into 4 chunks so it can spread queues.
            qtr = NC_NODES // 4  # 1562; remainder handled by last chunk
            for i in range(4):
                lo = i * qtr
                hi = NC_NODES if i == 3 else (i + 1) * qtr
                nc.sync.dma_start(sb_embT[:, lo:hi], embT_d[:, lo:hi])
            nc.sync.dma_start(sb_cwT[:], cwT_d[:])
            # explicit fp32 -> fp32r rounding for the TensorE inputs
            for i in range(4):
                lo = i * qtr
                hi = NC_NODES if i == 3 else (i + 1) * qtr
                nc.vector.tensor_copy(sb_embTr[:, lo:hi], sb_embT[:, lo:hi])
            nc.vector.tensor_copy(sb_cwTr[:], sb_cwT[:])
            nc.sync.dma_start(sb_par[:], par_d[:])
            nc.sync.dma_start(sb_rtpar[:], rtpar_d[:])
            nc.sync.dma_start(sb_consts[:], consts_d[:])
            nc.sync.dma_start(sb_tdc[:], tdc_d[:])
            nc.sync.dma_start(sb_tdb[:], tdb_d[:])

            ident = sb_consts[:, 0:128]  # eye(128)
            ones = sb_consts[:, 128:256]  # all-ones
            ltri = sb_consts[:, 256:384]  # ltri[p, f] = 1 if p <= f

            # ========== return_time_pred branch (64 events) ==========
            with (
                tc.tile_pool(name="rt_sb", bufs=1) as rpool,
                tc.tile_pool(name="rt_ps", bufs=4, space="PSUM") as rps,
            ):
                rps2 = rps
                ev = NC_EV
                E = rpool.tile([ev, S], dt.float32)
                Ften = rpool.tile([ev, S], dt.float32)
                I = rpool.tile([ev, S], dt.float32)
                It = rpool.tile([128, S_BLKS * ev], dt.float32)
                X = rpool.tile([128, S_BLKS * ev], dt.float32)
                TS = rpool.tile([128, S_BLKS * ev], dt.float32)
                rtp_sb = rpool.tile([1, ev], dt.float32)

                # E = exp(-w_t/1e4 * s)
                nc.scalar.activation(E[:], sb_tdb[:], AF.Exp, scale=sb_rtpar[:, 2:3])
                # F = exp(rt_scale*E + rt_bias)
                nc.scalar.activation(
                    Ften[:], E[:], AF.Exp,
                    scale=sb_rtpar[:, 1:2], bias=sb_rtpar[:, 0:1],
                )
                # I = psi * ln(F + 1)
                nc.scalar.activation(I[:], Ften[:], AF.Ln, bias=1.0)
                nc.vector.tensor_scalar_mul(I[:], I[:], sb_rtpar[:, 3:4])

                # transpose I to (s, b) layout, 128-row blocks
                for blk in range(S_BLKS):
                    pr = 128 if blk < 7 else LAST_BLK_ROWS
                    pst = rps.tile([128, ev], dt.float32, tag="rtps")
                    nc.tensor.transpose(
                        pst[0:pr, :], I[:, blk * 128 : blk * 128 + pr], ident[0:ev, 0:ev]
                    )
                    nc.vector.tensor_copy(
                        It[0:pr, blk * ev : (blk + 1) * ev], pst[0:pr, :]
                    )

                # cumsum over s (inclusive) via ones/ltri matmuls, then
                # X = exp(-TIMESTEP * cum), density, ts
                for r in range(S_BLKS):
                    pr = 128 if r < 7 else LAST_BLK_ROWS
                    psc = rps2.tile([128, ev], dt.float32, tag="rtps")
                    for k in range(r + 1):
                        pk = 128 if k < 7 else LAST_BLK_ROWS
                        if k < r:
                            lhs = ones[0:pk, 0:pr]
                        else:
                            lhs = ltri[0:pk, 0:pr]
                        nc.tensor.matmul(
                            psc[0:pr, :],
                            lhs,
                            It[0:pk, k * ev : (k + 1) * ev],
                            start=(k == 0),
                            stop=(k == r),
                        )
                    nc.scalar.activation(
                        X[0:pr, r * ev : (r + 1) * ev], psc[0:pr, :], AF.Exp,
                        scale=-TIMESTEP,
                    )
                    # density = I_T * X ; ts = density * td_s
                    nc.vector.tensor_tensor(
                        TS[0:pr, r * ev : (r + 1) * ev],
                        X[0:pr, r * ev : (r + 1) * ev],
                        It[0:pr, r * ev : (r + 1) * ev],
                        mybir.AluOpType.mult,
                    )
                    nc.vector.tensor_scalar_mul(
                        TS[0:pr, r * ev : (r + 1) * ev],
                        TS[0:pr, r * ev : (r + 1) * ev],
                        sb_tdc[0:pr, r : r + 1],
                    )

                # trapezoid-weighted sum over s via accumulating matmuls:
                # rtp[b] = sum_s w_s * ts[s, b] with w in tdc cols 8..15
                pss = rps.tile([1, ev], dt.float32, tag="rtps")
                for r in range(S_BLKS):
                    pr = 128 if r < 7 else LAST_BLK_ROWS
                    nc.tensor.matmul(
                        pss[0:1, :],
                        sb_tdc[0:pr, S_BLKS + r : S_BLKS + r + 1],
                        TS[0:pr, r * ev : (r + 1) * ev],
                        start=(r == 0),
                        stop=(r == S_BLKS - 1),
                    )
                nc.vector.tensor_copy(rtp_sb[:], pss[0:1, :])
                nc.sync.dma_start(rtp_d[:], rtp_sb[:])

            # ========== lambda_src / lambda_dst main loop ==========
            CH = 2048
            chunks = []
            off = 0
            while off < NC_NODES:
                chunks.append((off, min(CH, NC_NODES - off)))
                off += CH

            with (
                tc.tile_pool(name="lam_ps", bufs=2, space="PSUM") as lps,
                tc.tile_pool(name="lam_e", bufs=3) as epool,
                tc.tile_pool(name="lam_o", bufs=3) as opool,
            ):
                for off, F in chunks:
                    for tg in range(8):
                        g = tg % 4
                        eb = epool.tile([128, CH], dt.float32)
                        for s0 in range(0, F, 1024):
                            ss1 = min(1024, F - s0)
                            ps = lps.tile([128, 1024], dt.float32, tag="lps")
                            for s1 in range(0, ss1, 512):
                                ss = min(512, ss1 - s1)
                                nc.tensor.matmul(
                                    ps[:, s1 : s1 + ss],
                                    sb_cwTr[:, tg * 128 : (tg + 1) * 128],
                                    sb_embTr[:, off + s0 + s1 : off + s0 + s1 + ss],
                                    start=True,
                                    stop=True,
                                )
                            nc.scalar.activation(
                                eb[:, s0 : s0 + ss1], ps[:, 0:ss1], AF.Exp,
                                bias=sb_par[:, tg : tg + 1],
                            )
                        ob = opool.tile([128, CH], dt.float32)
                        nc.scalar.activation(ob[:, 0:F], eb[:, 0:F], AF.Ln, bias=1.0)
                        nc.vector.tensor_scalar_mul(
                            ob[:, 0:F], ob[:, 0:F], sb_par[:, 8 + g : 9 + g]
                        )
                        nc.sync.dma_start(
                            lam_d[tg * 128 : (tg + 1) * 128, off : off + F], ob[:, 0:F]
                        )

    nc.compile()
    return nc


def _get_program():
    if "nc" not in _PROGRAM_CACHE:
        _PROGRAM_CACHE["nc"] = _build_program()
    return _PROGRAM_CACHE["nc"]


def _host_prep(all_embeddings, assoc, src, pos_dst, last_update, cur_time, et,
               W, b, psi, alpha, w_t):
    """Per-event scalar prep (O(B*D)) + shard layouts. float64 intermediate
    for the tiny scalar math, cast to float32 for upload."""
    emb = np.asarray(all_embeddings, dtype=np.float32)
    assoc = np.asarray(assoc).astype(np.int64)
    src = np.asarray(src).astype(np.int64)
    pos_dst = np.asarray(pos_dst).astype(np.int64)
    lu = np.asarray(last_update, dtype=np.float64)
    ct = np.asarray(cur_time, dtype=np.float64)
    e = np.asarray(et).astype(np.int64)
    e = (e > 0).astype(np.int64)
    W = np.asarray(W, dtype=np.float64)
    bb = np.asarray(b, dtype=np.float64)
    psi = np.asarray(psi, dtype=np.float64)
    alpha = np.asarray(alpha, dtype=np.float64)
    w_t = np.asarray(w_t, dtype=np.float64)

    Wu = W[:, :D]  # (2, D)
    Wv = W[:, D:]

    idx_s = assoc[src]
    idx_d = assoc[pos_dst]
    zs = emb[idx_s].astype(np.float64)  # (B, D)
    zd = emb[idx_d].astype(np.float64)
    td_s = ct - lu[idx_s]
    td_d = ct - lu[idx_d]

    invpsi = 1.0 / (psi + 1e-7)
    ip = invpsi[e]  # (B,)
    psi_e = psi[e]
    alpha_e = alpha[e]
    wt_e = w_t[e]
    b_e = bb[e]

    a_s = np.einsum("bk,bk->b", zs, Wu[e])
    a_d = np.einsum("bk,bk->b", zd, Wv[e])
    wb_s = ip * (a_s + b_e + alpha_e * np.exp(-wt_e * td_s / TRAIN_TD_MAX))
    wb_d = ip * (a_d + b_e + alpha_e * np.exp(-wt_e * td_d / TRAIN_TD_MAX))

    cw_s = ip[:, None] * Wv[e]  # (B, D)   lambda_src node side goes via Wv
    cw_d = ip[:, None] * Wu[e]  # lambda_dst node side via Wu

    # cwT (D, 1024): col block tg = t*4+g holds events g*128..(g+1)*128 of type t
    cwT = np.zeros((D, 1024), dtype=np.float32)
    par = np.zeros((128, 12), dtype=np.float32)
    for g in range(4):
        sl = slice(g * 128, (g + 1) * 128)
        cwT[:, (0 * 4 + g) * 128 : (0 * 4 + g + 1) * 128] = cw_s[sl].T
        cwT[:, (1 * 4 + g) * 128 : (1 * 4 + g + 1) * 128] = cw_d[sl].T
        par[:, 0 * 4 + g] = wb_s[sl]
        par[:, 1 * 4 + g] = wb_d[sl]
        par[:, 8 + g] = psi_e[sl]

    # rtp per-event scalars
    base = a_s + np.einsum("bk,bk->b", zd, Wv[e]) + b_e
    rt_bias = ip * base
    rt_scale = ip * alpha_e
    nws = -wt_e * (TIMESTEP / TRAIN_TD_MAX)  # exp(nws * s), s integer
    rtpar_full = np.stack(
        [rt_bias, rt_scale, nws, psi_e], axis=1
    ).astype(np.float32)  # (B, 4)

    # consts: [identity | ones | ltri]
    consts = np.zeros((128, 384), dtype=np.float32)
    consts[:, 0:128] = np.eye(128, dtype=np.float32)
    consts[:, 128:256] = 1.0
    pidx = np.arange(128)
    consts[:, 256:384] = (pidx[:, None] <= pidx[None, :]).astype(np.float32)

    # td per (partition, block): 0.01 * s (0 beyond s=1000); trapezoid
    # weights in cols 8..15: 0.01 for s<1000, 0.005 at s=1000, 0 beyond
    tdc = np.zeros((128, 2 * S_BLKS), dtype=np.float32)
    for blk in range(S_BLKS):
        s_vals = blk * 128 + pidx
        valid = s_vals < S
        tdc[valid, blk] = (TIMESTEP * s_vals[valid]).astype(np.float32)
        w = np.where(s_vals < S - 1, TIMESTEP, np.where(s_vals == S - 1, 0.5 * TIMESTEP, 0.0))
        tdc[:, S_BLKS + blk] = w.astype(np.float32)

    # s-grid broadcast to all 64 event partitions
    tdb = np.broadcast_to(
        np.arange(S, dtype=np.float32)[None, :], (NC_EV, S)
    ).copy()

    # per-core embT slices (pre-transposed layout)
    embT_slices = []
    for c in range(NCORES):
        sl = emb[c * NC_NODES : (c + 1) * NC_NODES, :]
        embT_slices.append(np.ascontiguousarray(sl.T))

    rtpar_slices = [
        np.ascontiguousarray(rtpar_full[c * NC_EV : (c + 1) * NC_EV])
        for c in range(NCORES)
    ]

    return cwT, par, consts, tdc, tdb, embT_slices, rtpar_slices


def kernel(all_embeddings, assoc, src, pos_dst, neg_dst, last_update,
           cur_time, et, W, b, psi, alpha, w_t):
    from concourse.bass_utils import run_bass_kernel_spmd

    cwT, par, consts, tdc, tdb, embT_slices, rtpar_slices = _host_prep(
        all_embeddings, assoc, src, pos_dst, last_update, cur_time, et,
        W, b, psi, alpha, w_t,
    )

    nc = _get_program()

    in_maps = []
    for c in range(NCORES):
        in_maps.append({
            "embT": embT_slices[c],
            "cwT": cwT,
            "par": par,
            "rtpar": rtpar_slices[c],
            "consts": consts,
            "tdc": tdc,
            "tdb": tdb,
        })

    res = run_bass_kernel_spmd(nc, in_maps, core_ids=list(range(NCORES))).results

    lam_parts = [res[c]["lam"] for c in range(NCORES)]  # (1024, 6250) each
    lambda_src = np.concatenate([p[:512] for p in lam_parts], axis=1)
    lambda_dst = np.concatenate([p[512:] for p in lam_parts], axis=1)
    rtp = np.concatenate([res[c]["rtp"].reshape(NC_EV) for c in range(NCORES)])
    return lambda_src, lambda_dst, rtp


# revision 17
# speedup vs baseline: 1.0749x; 1.0749x over previous
"""Trainium2 Bass kernel for nn_Decoder (Hawkes intensity decoder).

Contract: kernel(**inputs) takes FULL unsharded inputs (as produced by the
reference's setup_inputs) and returns the full (lambda_src, lambda_dst,
return_time_pred) tuple.

Sharding (8 NeuronCores):
  - lambda_src/lambda_dst (B=512 x N=50000): node-sharded. Core c computes
    ALL 512 events against its 6250-node slice of all_embeddings. This cuts
    the per-core input DMA 8x vs batch-sharding (only the node slice is
    read) while output DMA (the roofline: 2 x 102.4MB fp32) is identical.
  - return_time_pred (B=512): batch-sharded, 64 events per core.

Per-core math. hawkes_intensity separates per (event b, node n):
    g[b,n] = z_ev[b].Wa[et_b] + emb[n].Wb[et_b] + bias[et_b]
             + alpha[et_b]*exp(-w_t[et_b]*td_b/100)
so with x = g/(psi+1e-7),
    lambda[b,n] = psi_e*(logaddexp(0,-x)+x) = psi_e*ln(1+exp(x)).
The node-independent part folds into a per-event scalar w_b; the node part
is a K=32 matmul row: x[b,n] = CW[b,:].emb[n,:] + w_b with
CW[b,:] = Wb[et_b]/(psi_e+1e-7). On device:
    PE   : PSUM[128ev, n] = cwT^T @ embT     (fp32r, K=32)
    ACT  : e = Exp(PSUM + w_b)  (per-partition bias)
    ACT  : l = Ln(e + 1)
    DVE  : out = l * psi_b      (per-partition scalar)
    DMA  : out -> lam[...]
|x| <~ 25 for any plausible input here, so Ln(1+Exp(x)) is overflow-safe in
fp32 and equals the reference's stable logaddexp form.

return_time_pred per core (64 events):
    E[b,s] = Exp(s * (-w_t_b*1e-4))         (s = 0..1000 row from host)
    F = Exp(rt_scale_b*E + rt_bias_b); I = psi_b*Ln(F+1)      # intensity
    transpose I -> (s,b); cumsum over s via triangular/ones matmuls
    X = Exp(-0.01*cum); ts = I_T*X*td_s; rtp = 0.01*sum(ts) - 0.005*ts[1000]
"""

import numpy as np

N_NODES = 50000
B = 512
D = 32
NCORES = 8
NC_NODES = N_NODES // NCORES  # 6250
NC_EV = B // NCORES  # 64
S = 1001
S_BLKS = 8  # ceil(1001/128); last block has 105 rows
LAST_BLK_ROWS = S - 7 * 128  # 105
TRAIN_TD_MAX = 100.0
TIMESTEP = 0.01

_PROGRAM_CACHE = {}


def _build_program():
    """Build + compile the SPMD Bass program (identical on all 8 cores)."""
    import concourse.bass as bass
    import concourse.mybir as mybir
    from concourse import bacc, tile

    dt = mybir.dt
    AF = mybir.ActivationFunctionType

    # Both Exp and Ln live in the 'natural_log_exp_and_others' activation
    # table set; left to itself the table-load pass picks per-function sets
    # and the Scalar engine reloads tables on every Exp<->Ln alternation
    # (~1.3us each, ~46us total here). Restrict selection to the shared set
    # (other sets are emptied, keeping dict order so act_func_set_id indices
    # stay aligned with act_info.json).
    from concourse.hw_specs import get_activation_tables as _real_gat

    def _patched_gat(arch):
        tabs = _real_gat(arch)
        return {
            k: (v if k == "natural_log_exp_and_others" else set())
            for k, v in tabs.items()
        }

    bacc.get_activation_tables = _patched_gat

    nc = bacc.Bacc(
        "TRN2",
        target_bir_lowering=False,
        debug=False,
        num_devices=NCORES,
    )

    # ---- DRAM parameters -------------------------------------------------
    embT_d = nc.declare_dram_parameter("embT", [D, NC_NODES], dt.float32, isOutput=False)
    cwT_d = nc.declare_dram_parameter("cwT", [D, 1024], dt.float32, isOutput=False)
    par_d = nc.declare_dram_parameter("par", [128, 12], dt.float32, isOutput=False)
    rtpar_d = nc.declare_dram_parameter("rtpar", [NC_EV, 4], dt.float32, isOutput=False)
    consts_d = nc.declare_dram_parameter("consts", [128, 384], dt.float32, isOutput=False)
    # tdc cols 0..7: td value 0.01*s per (partition, block); cols 8..15:
    # trapezoid weights (0.01 for s<1000, 0.005 at s=1000, 0 beyond)
    tdc_d = nc.declare_dram_parameter("tdc", [128, 2 * S_BLKS], dt.float32, isOutput=False)
    tdb_d = nc.declare_dram_parameter("tdb", [NC_EV, S], dt.float32, isOutput=False)

    lam_d = nc.declare_dram_parameter("lam", [1024, NC_NODES], dt.float32, isOutput=True)
    rtp_d = nc.declare_dram_parameter("rtp", [1, NC_EV], dt.float32, isOutput=True)

    F32R = dt.float32r

    with tile.TileContext(nc) as tc:
        with tc.tile_pool(name="const", bufs=1) as cpool:
            sb_embT = cpool.tile([D, NC_NODES], dt.float32)
            sb_cwT = cpool.tile([D, 1024], dt.float32)
            sb_embTr = cpool.tile([D, NC_NODES], F32R)
            sb_cwTr = cpool.tile([D, 1024], F32R)
            sb_par = cpool.tile([128, 12], dt.float32)
            sb_rtpar = cpool.tile([NC_EV, 4], dt.float32)
            sb_consts = cpool.tile([128, 384], dt.float32)
            sb_tdc = cpool.tile([128, 2 * S_BLKS], dt.float32)
            sb_tdb = cpool.tile([NC_EV, S], dt.float32)

            nc.sync.dma_start(sb_cwT[:], cwT_d[:])
            nc.vector.tensor_copy(sb_cwTr[:], sb_cwT[:])
            nc.sync.dma_start(sb_par[:], par_d[:])
            nc.sync.dma_start(sb_rtpar[:], rtpar_d[:])
            nc.sync.dma_start(sb_consts[:], consts_d[:])
            nc.sync.dma_start(sb_tdc[:], tdc_d[:])
            nc.sync.dma_start(sb_tdb[:], tdb_d[:])
            # embT load + fp32r rounding, pipelined in 8 chunks so the first
            # lambda matmuls can start while the rest streams in.
            n_ld = 8
            ld = (NC_NODES + n_ld - 1) // n_ld
            for i in range(n_ld):
                lo = i * ld
                hi = min(NC_NODES, (i + 1) * ld)
                nc.sync.dma_start(sb_embT[:, lo:hi], embT_d[:, lo:hi])
                nc.vector.tensor_copy(sb_embTr[:, lo:hi], sb_embT[:, lo:hi])

            ident = sb_consts[:, 0:128]  # eye(128)
            ones = sb_consts[:, 128:256]  # all-ones
            ltri = sb_consts[:, 256:384]  # ltri[p, f] = 1 if p <= f

            # ========== return_time_pred: intensity I (head) ==========
            # Only the SBUF-resident intensity computation runs up front
            # (it needs no PSUM and fills the Scalar engine while embT
            # streams in). The PSUM-heavy cumsum/integral runs at the very
            # end, overlapping the final output-DMA drain.
            with tc.tile_pool(name="rt_sb", bufs=1) as rpool:
                ev = NC_EV
                E = rpool.tile([ev, S], dt.float32)
                Ften = rpool.tile([ev, S], dt.float32)
                I = rpool.tile([ev, S], dt.float32)
                It = rpool.tile([128, S_BLKS * ev], dt.float32)
                X = rpool.tile([128, S_BLKS * ev], dt.float32)
                TS = rpool.tile([128, S_BLKS * ev], dt.float32)
                rtp_sb = rpool.tile([1, ev], dt.float32)

                # E = exp(-w_t/1e4 * s)
                nc.scalar.activation(E[:], sb_tdb[:], AF.Exp, scale=sb_rtpar[:, 2:3])
                # F = exp(rt_scale*E + rt_bias)
                nc.scalar.activation(
                    Ften[:], E[:], AF.Exp,
                    scale=sb_rtpar[:, 1:2], bias=sb_rtpar[:, 0:1],
                )
                # I = psi * ln(F + 1)
                nc.scalar.activation(I[:], Ften[:], AF.Ln, bias=1.0)
                nc.vector.tensor_scalar_mul(I[:], I[:], sb_rtpar[:, 3:4])

                # ========== lambda_src / lambda_dst main loop ==========
                CH = 2048
                chunks = []
                off = 0
                while off < NC_NODES:
                    chunks.append((off, min(CH, NC_NODES - off)))
                    off += CH

                with (
                    tc.tile_pool(name="lam_ps", bufs=2, space="PSUM") as lps,
                    tc.tile_pool(name="lam_e", bufs=3) as epool,
                    tc.tile_pool(name="lam_o", bufs=3) as opool,
                ):
                    for off, F in chunks:
                        for tg in range(8):
                            g = tg % 4
                            ps = lps.tile([128, CH], dt.float32, tag="lps")
                            for s1 in range(0, F, 512):
                                ss = min(512, F - s1)
                                nc.tensor.matmul(
                                    ps[:, s1 : s1 + ss],
                                    sb_cwTr[:, tg * 128 : (tg + 1) * 128],
                                    sb_embTr[:, off + s1 : off + s1 + ss],
                                    start=True,
                                    stop=True,
                                )
                            eb = epool.tile([128, CH], dt.float32)
                            nc.scalar.activation(
                                eb[:, 0:F], ps[:, 0:F], AF.Exp,
                                bias=sb_par[:, tg : tg + 1],
                            )
                            ob = opool.tile([128, CH], dt.float32)
                            nc.scalar.activation(
                                ob[:, 0:F], eb[:, 0:F], AF.Ln, bias=1.0
                            )
                            nc.vector.tensor_scalar_mul(
                                ob[:, 0:F], ob[:, 0:F], sb_par[:, 8 + g : 9 + g]
                            )
                            nc.sync.dma_start(
                                lam_d[tg * 128 : (tg + 1) * 128, off : off + F],
                                ob[:, 0:F],
                            )

                # ========== return_time_pred: integral (tail) ==========
                with tc.tile_pool(name="rt_ps", bufs=4, space="PSUM") as rps:
                    # transpose I to (s, b) layout, 128-row blocks
                    for blk in range(S_BLKS):
                        pr = 128 if blk < 7 else LAST_BLK_ROWS
                        pst = rps.tile([128, ev], dt.float32, tag="rtps")
                        nc.tensor.transpose(
                            pst[0:pr, :], I[:, blk * 128 : blk * 128 + pr],
                            ident[0:ev, 0:ev],
                        )
                        nc.vector.tensor_copy(
                            It[0:pr, blk * ev : (blk + 1) * ev], pst[0:pr, :]
                        )

                    # inclusive cumsum over s via ones/ltri matmuls, then
                    # X = exp(-TIMESTEP * cum), density, ts
                    for r in range(S_BLKS):
                        pr = 128 if r < 7 else LAST_BLK_ROWS
                        psc = rps.tile([128, ev], dt.float32, tag="rtps")
                        for k in range(r + 1):
                            pk = 128 if k < 7 else LAST_BLK_ROWS
                            lhs = ones[0:pk, 0:pr] if k < r else ltri[0:pk, 0:pr]
                            nc.tensor.matmul(
                                psc[0:pr, :],
                                lhs,
                                It[0:pk, k * ev : (k + 1) * ev],
                                start=(k == 0),
                                stop=(k == r),
                            )
                        nc.scalar.activation(
                            X[0:pr, r * ev : (r + 1) * ev], psc[0:pr, :], AF.Exp,
                            scale=-TIMESTEP,
                        )
                        # density = I_T * X ; ts = density * td_s
                        nc.vector.tensor_tensor(
                            TS[0:pr, r * ev : (r + 1) * ev],
                            X[0:pr, r * ev : (r + 1) * ev],
                            It[0:pr, r * ev : (r + 1) * ev],
                            mybir.AluOpType.mult,
                        )
                        nc.vector.tensor_scalar_mul(
                            TS[0:pr, r * ev : (r + 1) * ev],
                            TS[0:pr, r * ev : (r + 1) * ev],
                            sb_tdc[0:pr, r : r + 1],
                        )

                    # trapezoid-weighted sum over s via accumulating matmuls:
                    # rtp[b] = sum_s w_s * ts[s, b] with w in tdc cols 8..15
                    pss = rps.tile([1, ev], dt.float32, tag="rtps")
                    for r in range(S_BLKS):
                        pr = 128 if r < 7 else LAST_BLK_ROWS
                        nc.tensor.matmul(
                            pss[0:1, :],
                            sb_tdc[0:pr, S_BLKS + r : S_BLKS + r + 1],
                            TS[0:pr, r * ev : (r + 1) * ev],
                            start=(r == 0),
                            stop=(r == S_BLKS - 1),
                        )
                    nc.vector.tensor_copy(rtp_sb[:], pss[0:1, :])
                    nc.sync.dma_start(rtp_d[:], rtp_sb[:])

    nc.compile()
    return nc


def _get_program():
    if "nc" not in _PROGRAM_CACHE:
        _PROGRAM_CACHE["nc"] = _build_program()
    return _PROGRAM_CACHE["nc"]


def _host_prep(all_embeddings, assoc, src, pos_dst, last_update, cur_time, et,
               W, b, psi, alpha, w_t):
    """Per-event scalar prep (O(B*D)) + shard layouts. float64 intermediate
    for the tiny scalar math, cast to float32 for upload."""
    emb = np.asarray(all_embeddings, dtype=np.float32)
    assoc = np.asarray(assoc).astype(np.int64)
    src = np.asarray(src).astype(np.int64)
    pos_dst = np.asarray(pos_dst).astype(np.int64)
    lu = np.asarray(last_update, dtype=np.float64)
    ct = np.asarray(cur_time, dtype=np.float64)
    e = np.asarray(et).astype(np.int64)
    e = (e > 0).astype(np.int64)
    W = np.asarray(W, dtype=np.float64)
    bb = np.asarray(b, dtype=np.float64)
    psi = np.asarray(psi, dtype=np.float64)
    alpha = np.asarray(alpha, dtype=np.float64)
    w_t = np.asarray(w_t, dtype=np.float64)

    Wu = W[:, :D]  # (2, D)
    Wv = W[:, D:]

    idx_s = assoc[src]
    idx_d = assoc[pos_dst]
    zs = emb[idx_s].astype(np.float64)  # (B, D)
    zd = emb[idx_d].astype(np.float64)
    td_s = ct - lu[idx_s]
    td_d = ct - lu[idx_d]

    invpsi = 1.0 / (psi + 1e-7)
    ip = invpsi[e]  # (B,)
    psi_e = psi[e]
    alpha_e = alpha[e]
    wt_e = w_t[e]
    b_e = bb[e]

    a_s = np.einsum("bk,bk->b", zs, Wu[e])
    a_d = np.einsum("bk,bk->b", zd, Wv[e])
    wb_s = ip * (a_s + b_e + alpha_e * np.exp(-wt_e * td_s / TRAIN_TD_MAX))
    wb_d = ip * (a_d + b_e + alpha_e * np.exp(-wt_e * td_d / TRAIN_TD_MAX))

    cw_s = ip[:, None] * Wv[e]  # (B, D)   lambda_src node side goes via Wv
    cw_d = ip[:, None] * Wu[e]  # lambda_dst node side via Wu

    # cwT (D, 1024): col block tg = t*4+g holds events g*128..(g+1)*128 of type t
    cwT = np.zeros((D, 1024), dtype=np.float32)
    par = np.zeros((128, 12), dtype=np.float32)
    for g in range(4):
        sl = slice(g * 128, (g + 1) * 128)
        cwT[:, (0 * 4 + g) * 128 : (0 * 4 + g + 1) * 128] = cw_s[sl].T
        cwT[:, (1 * 4 + g) * 128 : (1 * 4 + g + 1) * 128] = cw_d[sl].T
        par[:, 0 * 4 + g] = wb_s[sl]
        par[:, 1 * 4 + g] = wb_d[sl]
        par[:, 8 + g] = psi_e[sl]

    # rtp per-event scalars
    base = a_s + np.einsum("bk,bk->b", zd, Wv[e]) + b_e
    rt_bias = ip * base
    rt_scale = ip * alpha_e
    nws = -wt_e * (TIMESTEP / TRAIN_TD_MAX)  # exp(nws * s), s integer
    rtpar_full = np.stack(
        [rt_bias, rt_scale, nws, psi_e], axis=1
    ).astype(np.float32)  # (B, 4)

    # consts: [identity | ones | ltri]
    consts = np.zeros((128, 384), dtype=np.float32)
    consts[:, 0:128] = np.eye(128, dtype=np.float32)
    consts[:, 128:256] = 1.0
    pidx = np.arange(128)
    consts[:, 256:384] = (pidx[:, None] <= pidx[None, :]).astype(np.float32)

    # td per (partition, block): 0.01 * s (0 beyond s=1000); trapezoid
    # weights in cols 8..15: 0.01 for s<1000, 0.005 at s=1000, 0 beyond
    tdc = np.zeros((128, 2 * S_BLKS), dtype=np.float32)
    for blk in range(S_BLKS):
        s_vals = blk * 128 + pidx
        valid = s_vals < S
        tdc[valid, blk] = (TIMESTEP * s_vals[valid]).astype(np.float32)
        w = np.where(s_vals < S - 1, TIMESTEP, np.where(s_vals == S - 1, 0.5 * TIMESTEP, 0.0))
        tdc[:, S_BLKS + blk] = w.astype(np.float32)

    # s-grid broadcast to all 64 event partitions
    tdb = np.broadcast_to(
        np.arange(S, dtype=np.float32)[None, :], (NC_EV, S)
    ).copy()

    # per-core embT slices (pre-transposed layout)
    embT_slices = []
    for c in range(NCORES):
        sl = emb[c * NC_NODES : (c + 1) * NC_NODES, :]
        embT_slices.append(np.ascontiguousarray(sl.T))

    rtpar_slices = [
        np.ascontiguousarray(rtpar_full[c * NC_EV : (c + 1) * NC_EV])
        for c in range(NCORES)
    ]

    return cwT, par, consts, tdc, tdb, embT_slices, rtpar_slices


def kernel(all_embeddings, assoc, src, pos_dst, neg_dst, last_update,
           cur_time, et, W, b, psi, alpha, w_t):
    from concourse.bass_utils import run_bass_kernel_spmd

    cwT, par, consts, tdc, tdb, embT_slices, rtpar_slices = _host_prep(
        all_embeddings, assoc, src, pos_dst, last_update, cur_time, et,
        W, b, psi, alpha, w_t,
    )

    nc = _get_program()

    in_maps = []
    for c in range(NCORES):
        in_maps.append({
            "embT": embT_slices[c],
            "cwT": cwT,
            "par": par,
            "rtpar": rtpar_slices[c],
            "consts": consts,
            "tdc": tdc,
            "tdb": tdb,
        })

    res = run_bass_kernel_spmd(nc, in_maps, core_ids=list(range(NCORES))).results

    lam_parts = [res[c]["lam"] for c in range(NCORES)]  # (1024, 6250) each
    lambda_src = np.concatenate([p[:512] for p in lam_parts], axis=1)
    lambda_dst = np.concatenate([p[512:] for p in lam_parts], axis=1)
    rtp = np.concatenate([res[c]["rtp"].reshape(NC_EV) for c in range(NCORES)])
    return lambda_src, lambda_dst, rtp


# revision 18
# speedup vs baseline: 1.1825x; 1.1001x over previous
"""Trainium2 Bass kernel for nn_Decoder (Hawkes intensity decoder).

Contract: kernel(**inputs) takes FULL unsharded inputs (as produced by the
reference's setup_inputs) and returns the full (lambda_src, lambda_dst,
return_time_pred) tuple.

Sharding (8 NeuronCores):
  - lambda_src/lambda_dst (B=512 x N=50000): node-sharded. Core c computes
    ALL 512 events against its 6250-node slice of all_embeddings. This cuts
    the per-core input DMA 8x vs batch-sharding (only the node slice is
    read) while output DMA (the roofline: 2 x 102.4MB fp32) is identical.
  - return_time_pred (B=512): batch-sharded, 64 events per core.

Per-core math. hawkes_intensity separates per (event b, node n):
    g[b,n] = z_ev[b].Wa[et_b] + emb[n].Wb[et_b] + bias[et_b]
             + alpha[et_b]*exp(-w_t[et_b]*td_b/100)
so with x = g/(psi+1e-7),
    lambda[b,n] = psi_e*(logaddexp(0,-x)+x) = psi_e*ln(1+exp(x)).
The node-independent part folds into a per-event scalar w_b; the node part
is a K=32 matmul row: x[b,n] = CW[b,:].emb[n,:] + w_b with
CW[b,:] = Wb[et_b]/(psi_e+1e-7). On device:
    PE   : PSUM[128ev, n] = cwT^T @ embT     (fp32r, K=32)
    ACT  : e = Exp(PSUM + w_b)  (per-partition bias)
    ACT  : l = Ln(e + 1)
    DVE  : out = l * psi_b      (per-partition scalar)
    DMA  : out -> lam[...]
|x| <~ 25 for any plausible input here, so Ln(1+Exp(x)) is overflow-safe in
fp32 and equals the reference's stable logaddexp form.

return_time_pred per core (64 events on partitions, s=0..1000 on the free
dim; tdb2 carries [s grid | trapezoid-weight*td]):
    E = Exp(s * (-w_t_b*1e-4)); F = Exp(rt_scale_b*E + rt_bias_b)
    I = psi_b*Ln(F+1)                       # intensity (b, s)
    cum = tensor_tensor_scan(I, add)        # inclusive cumsum along s (DVE)
    X = Exp(-0.01*cum); density = I*X
    rtp = reduce_sum(density * wtd, axis=s) # wtd = trapz weight * td
"""

import numpy as np

N_NODES = 50000
B = 512
D = 32
NCORES = 8
NC_NODES = N_NODES // NCORES  # 6250
NC_EV = B // NCORES  # 64
S = 1001
TRAIN_TD_MAX = 100.0
TIMESTEP = 0.01

_PROGRAM_CACHE = {}


def _build_program():
    """Build + compile the SPMD Bass program (identical on all 8 cores)."""
    import concourse.bass as bass
    import concourse.mybir as mybir
    from concourse import bacc, tile

    dt = mybir.dt
    AF = mybir.ActivationFunctionType

    # Both Exp and Ln live in the 'natural_log_exp_and_others' activation
    # table set; left to itself the table-load pass picks per-function sets
    # and the Scalar engine reloads tables on every Exp<->Ln alternation
    # (~1.3us each, ~46us total here). Restrict selection to the shared set
    # (other sets are emptied, keeping dict order so act_func_set_id indices
    # stay aligned with act_info.json).
    from concourse.hw_specs import get_activation_tables as _real_gat

    def _patched_gat(arch):
        tabs = _real_gat(arch)
        return {
            k: (v if k == "natural_log_exp_and_others" else set())
            for k, v in tabs.items()
        }

    bacc.get_activation_tables = _patched_gat

    nc = bacc.Bacc(
        "TRN2",
        target_bir_lowering=False,
        debug=False,
        num_devices=NCORES,
    )

    # ---- DRAM parameters -------------------------------------------------
    embT_d = nc.declare_dram_parameter("embT", [D, NC_NODES], dt.float32, isOutput=False)
    cwT_d = nc.declare_dram_parameter("cwT", [D, 1024], dt.float32, isOutput=False)
    par_d = nc.declare_dram_parameter("par", [128, 12], dt.float32, isOutput=False)
    rtpar_d = nc.declare_dram_parameter("rtpar", [NC_EV, 4], dt.float32, isOutput=False)
    tdb_d = nc.declare_dram_parameter("tdb", [NC_EV, 2 * S], dt.float32, isOutput=False)

    lam_d = nc.declare_dram_parameter("lam", [1024, NC_NODES], dt.float32, isOutput=True)
    rtp_d = nc.declare_dram_parameter("rtp", [NC_EV, 1], dt.float32, isOutput=True)

    F32R = dt.float32r

    with tile.TileContext(nc) as tc:
        with tc.tile_pool(name="const", bufs=1) as cpool:
            sb_embT = cpool.tile([D, NC_NODES], dt.float32)
            sb_cwT = cpool.tile([D, 1024], dt.float32)
            sb_embTr = cpool.tile([D, NC_NODES], F32R)
            sb_cwTr = cpool.tile([D, 1024], F32R)
            sb_par = cpool.tile([128, 12], dt.float32)
            sb_rtpar = cpool.tile([NC_EV, 4], dt.float32)
            sb_tdb = cpool.tile([NC_EV, 2 * S], dt.float32)

            nc.sync.dma_start(sb_tdb[:], tdb_d[:])
            nc.sync.dma_start(sb_rtpar[:], rtpar_d[:])
            nc.sync.dma_start(sb_par[:], par_d[:])
            nc.sync.dma_start(sb_cwT[:], cwT_d[:])
            nc.vector.tensor_copy(sb_cwTr[:], sb_cwT[:])
            # embT load + fp32r rounding, pipelined in 8 chunks so the first
            # lambda matmuls can start while the rest streams in.
            n_ld = 8
            ld = (NC_NODES + n_ld - 1) // n_ld
            for i in range(n_ld):
                lo = i * ld
                hi = min(NC_NODES, (i + 1) * ld)
                nc.sync.dma_start(sb_embT[:, lo:hi], embT_d[:, lo:hi])
                nc.vector.tensor_copy(sb_embTr[:, lo:hi], sb_embT[:, lo:hi])

            # ========== return_time_pred (64 events, no PSUM) ==========
            with tc.tile_pool(name="rt_sb", bufs=1) as rpool:
                ev = NC_EV
                E = rpool.tile([ev, S], dt.float32)
                Ften = rpool.tile([ev, S], dt.float32)
                I = rpool.tile([ev, S], dt.float32)
                cum = rpool.tile([ev, S], dt.float32)
                dens = rpool.tile([ev, S], dt.float32)
                rtp_sb = rpool.tile([ev, 1], dt.float32)

                sgrid = sb_tdb[:, 0:S]
                wtd = sb_tdb[:, S : 2 * S]

                # E = exp(-w_t/1e4 * s)
                nc.scalar.activation(E[:], sgrid, AF.Exp, scale=sb_rtpar[:, 2:3])
                # F = exp(rt_scale*E + rt_bias)
                nc.scalar.activation(
                    Ften[:], E[:], AF.Exp,
                    scale=sb_rtpar[:, 1:2], bias=sb_rtpar[:, 0:1],
                )
                # I = psi * ln(F + 1)
                nc.scalar.activation(I[:], Ften[:], AF.Ln, bias=1.0)
                nc.vector.tensor_scalar_mul(I[:], I[:], sb_rtpar[:, 3:4])
                # inclusive cumsum along s (fp32 state)
                nc.vector.tensor_tensor_scan(
                    cum[:], I[:], I[:], 0.0,
                    mybir.AluOpType.add, mybir.AluOpType.bypass,
                )
                # X = exp(-TIMESTEP*cum); density = I * X (reuse cum tile)
                nc.scalar.activation(cum[:], cum[:], AF.Exp, scale=-TIMESTEP)
                nc.vector.tensor_tensor(
                    dens[:], cum[:], I[:], mybir.AluOpType.mult
                )
                # ts = density * (trapz_w * td); rtp = sum_s ts
                nc.vector.tensor_tensor(
                    dens[:], dens[:], wtd, mybir.AluOpType.mult
                )
                nc.vector.tensor_reduce(
                    rtp_sb[:], dens[:], mybir.AxisListType.X, mybir.AluOpType.add
                )
                nc.sync.dma_start(rtp_d[:], rtp_sb[:])

                # ========== lambda_src / lambda_dst main loop ==========
                CH = 2048
                chunks = []
                off = 0
                while off < NC_NODES:
                    chunks.append((off, min(CH, NC_NODES - off)))
                    off += CH

                with (
                    tc.tile_pool(name="lam_ps", bufs=2, space="PSUM") as lps,
                    tc.tile_pool(name="lam_e", bufs=3) as epool,
                    tc.tile_pool(name="lam_o", bufs=3) as opool,
                ):
                    for off, F in chunks:
                        for tg in range(8):
                            g = tg % 4
                            ps = lps.tile([128, CH], dt.float32, tag="lps")
                            for s1 in range(0, F, 512):
                                ss = min(512, F - s1)
                                nc.tensor.matmul(
                                    ps[:, s1 : s1 + ss],
                                    sb_cwTr[:, tg * 128 : (tg + 1) * 128],
                                    sb_embTr[:, off + s1 : off + s1 + ss],
                                    start=True,
                                    stop=True,
                                )
                            eb = epool.tile([128, CH], dt.float32)
                            nc.scalar.activation(
                                eb[:, 0:F], ps[:, 0:F], AF.Exp,
                                bias=sb_par[:, tg : tg + 1],
                            )
                            ob = opool.tile([128, CH], dt.float32)
                            nc.scalar.activation(
                                ob[:, 0:F], eb[:, 0:F], AF.Ln, bias=1.0
                            )
                            nc.vector.tensor_scalar_mul(
                                ob[:, 0:F], ob[:, 0:F], sb_par[:, 8 + g : 9 + g]
                            )
                            nc.sync.dma_start(
                                lam_d[tg * 128 : (tg + 1) * 128, off : off + F],
                                ob[:, 0:F],
                            )

    nc.compile()
    return nc


def _get_program():
    if "nc" not in _PROGRAM_CACHE:
        _PROGRAM_CACHE["nc"] = _build_program()
    return _PROGRAM_CACHE["nc"]


def _host_prep(all_embeddings, assoc, src, pos_dst, last_update, cur_time, et,
               W, b, psi, alpha, w_t):
    """Per-event scalar prep (O(B*D)) + shard layouts. float64 intermediate
    for the tiny scalar math, cast to float32 for upload."""
    emb = np.asarray(all_embeddings, dtype=np.float32)
    assoc = np.asarray(assoc).astype(np.int64)
    src = np.asarray(src).astype(np.int64)
    pos_dst = np.asarray(pos_dst).astype(np.int64)
    lu = np.asarray(last_update, dtype=np.float64)
    ct = np.asarray(cur_time, dtype=np.float64)
    e = np.asarray(et).astype(np.int64)
    e = (e > 0).astype(np.int64)
    W = np.asarray(W, dtype=np.float64)
    bb = np.asarray(b, dtype=np.float64)
    psi = np.asarray(psi, dtype=np.float64)
    alpha = np.asarray(alpha, dtype=np.float64)
    w_t = np.asarray(w_t, dtype=np.float64)

    Wu = W[:, :D]  # (2, D)
    Wv = W[:, D:]

    idx_s = assoc[src]
    idx_d = assoc[pos_dst]
    zs = emb[idx_s].astype(np.float64)  # (B, D)
    zd = emb[idx_d].astype(np.float64)
    td_s = ct - lu[idx_s]
    td_d = ct - lu[idx_d]

    invpsi = 1.0 / (psi + 1e-7)
    ip = invpsi[e]  # (B,)
    psi_e = psi[e]
    alpha_e = alpha[e]
    wt_e = w_t[e]
    b_e = bb[e]

    a_s = np.einsum("bk,bk->b", zs, Wu[e])
    a_d = np.einsum("bk,bk->b", zd, Wv[e])
    wb_s = ip * (a_s + b_e + alpha_e * np.exp(-wt_e * td_s / TRAIN_TD_MAX))
    wb_d = ip * (a_d + b_e + alpha_e * np.exp(-wt_e * td_d / TRAIN_TD_MAX))

    cw_s = ip[:, None] * Wv[e]  # (B, D)   lambda_src node side goes via Wv
    cw_d = ip[:, None] * Wu[e]  # lambda_dst node side via Wu

    # cwT (D, 1024): col block tg = t*4+g holds events g*128..(g+1)*128 of type t
    cwT = np.zeros((D, 1024), dtype=np.float32)
    par = np.zeros((128, 12), dtype=np.float32)
    for g in range(4):
        sl = slice(g * 128, (g + 1) * 128)
        cwT[:, (0 * 4 + g) * 128 : (0 * 4 + g + 1) * 128] = cw_s[sl].T
        cwT[:, (1 * 4 + g) * 128 : (1 * 4 + g + 1) * 128] = cw_d[sl].T
        par[:, 0 * 4 + g] = wb_s[sl]
        par[:, 1 * 4 + g] = wb_d[sl]
        par[:, 8 + g] = psi_e[sl]

    # rtp per-event scalars
    base = a_s + np.einsum("bk,bk->b", zd, Wv[e]) + b_e
    rt_bias = ip * base
    rt_scale = ip * alpha_e
    nws = -wt_e * (TIMESTEP / TRAIN_TD_MAX)  # exp(nws * s), s integer
    rtpar_full = np.stack(
        [rt_bias, rt_scale, nws, psi_e], axis=1
    ).astype(np.float32)  # (B, 4)

    # tdb2 (64, 2S): [s grid | trapezoid_weight * td]
    s_vals = np.arange(S, dtype=np.float64)
    w = np.full(S, TIMESTEP)
    w[-1] = 0.5 * TIMESTEP
    wtd = (w * (TIMESTEP * s_vals)).astype(np.float32)
    tdb2 = np.zeros((NC_EV, 2 * S), dtype=np.float32)
    tdb2[:, 0:S] = s_vals.astype(np.float32)[None, :]
    tdb2[:, S:] = wtd[None, :]

    # per-core embT slices (pre-transposed layout)
    embT_slices = []
    for c in range(NCORES):
        sl = emb[c * NC_NODES : (c + 1) * NC_NODES, :]
        embT_slices.append(np.ascontiguousarray(sl.T))

    rtpar_slices = [
        np.ascontiguousarray(rtpar_full[c * NC_EV : (c + 1) * NC_EV])
        for c in range(NCORES)
    ]

    return cwT, par, tdb2, embT_slices, rtpar_slices


def kernel(all_embeddings, assoc, src, pos_dst, neg_dst, last_update,
           cur_time, et, W, b, psi, alpha, w_t):
    from concourse.bass_utils import run_bass_kernel_spmd

    cwT, par, tdb2, embT_slices, rtpar_slices = _host_prep(
        all_embeddings, assoc, src, pos_dst, last_update, cur_time, et,
        W, b, psi, alpha, w_t,
    )

    nc = _get_program()

    in_maps = []
    for c in range(NCORES):
        in_maps.append({
            "embT": embT_slices[c],
            "cwT": cwT,
            "par": par,
            "rtpar": rtpar_slices[c],
            "tdb": tdb2,
        })

    res = run_bass_kernel_spmd(nc, in_maps, core_ids=list(range(NCORES))).results

    lam_parts = [res[c]["lam"] for c in range(NCORES)]  # (1024, 6250) each
    lambda_src = np.concatenate([p[:512] for p in lam_parts], axis=1)
    lambda_dst = np.concatenate([p[512:] for p in lam_parts], axis=1)
    rtp = np.concatenate([res[c]["rtp"].reshape(NC_EV) for c in range(NCORES)])
    return lambda_src, lambda_dst, rtp


# revision 22
# speedup vs baseline: 1.1964x; 1.0117x over previous
"""Trainium2 Bass kernel for nn_Decoder (Hawkes intensity decoder).

Contract: kernel(**inputs) takes FULL unsharded inputs (as produced by the
reference's setup_inputs) and returns the full (lambda_src, lambda_dst,
return_time_pred) tuple.

Sharding (8 NeuronCores):
  - lambda_src/lambda_dst (B=512 x N=50000): node-sharded. Core c computes
    ALL 512 events against its 6250-node slice of all_embeddings. This cuts
    the per-core input DMA 8x vs batch-sharding (only the node slice is
    read) while output DMA (the roofline: 2 x 102.4MB fp32) is identical.
  - return_time_pred (B=512): batch-sharded, 64 events per core.

Per-core math. hawkes_intensity separates per (event b, node n):
    g[b,n] = z_ev[b].Wa[et_b] + emb[n].Wb[et_b] + bias[et_b]
             + alpha[et_b]*exp(-w_t[et_b]*td_b/100)
so with x = g/(psi+1e-7),
    lambda[b,n] = psi_e*(logaddexp(0,-x)+x) = psi_e*ln(1+exp(x)).
The node-independent part folds into a per-event scalar w_b; the node part
is a K=32 matmul row: x[b,n] = CW[b,:].emb[n,:] + w_b with
CW[b,:] = Wb[et_b]/(psi_e+1e-7). On device:
    PE   : PSUM[128ev, n] = cwT^T @ embT     (fp32r, K=32)
    ACT  : e = Exp(PSUM + w_b)  (per-partition bias)
    ACT  : l = Ln(e + 1)
    DVE  : out = l * psi_b      (per-partition scalar)
    DMA  : out -> lam[...]
|x| <~ 25 for any plausible input here, so Ln(1+Exp(x)) is overflow-safe in
fp32 and equals the reference's stable logaddexp form.

return_time_pred per core (64 events on partitions, s=0..1000 on the free
dim; tdb2 carries [s grid | trapezoid-weight*td]):
    E = Exp(s * (-w_t_b*1e-4)); F = Exp(rt_scale_b*E + rt_bias_b)
    I = psi_b*Ln(F+1)                       # intensity (b, s)
    cum = tensor_tensor_scan(I, add)        # inclusive cumsum along s (DVE)
    X = Exp(-0.01*cum); density = I*X
    rtp = reduce_sum(density * wtd, axis=s) # wtd = trapz weight * td
"""

import numpy as np

N_NODES = 50000
B = 512
D = 32
NCORES = 8
NC_NODES = N_NODES // NCORES  # 6250
NC_EV = B // NCORES  # 64
S = 1001
TRAIN_TD_MAX = 100.0
TIMESTEP = 0.01

_PROGRAM_CACHE = {}


def _build_program():
    """Build + compile the SPMD Bass program (identical on all 8 cores)."""
    import concourse.bass as bass
    import concourse.mybir as mybir
    from concourse import bacc, tile

    dt = mybir.dt
    AF = mybir.ActivationFunctionType

    # Both Exp and Ln live in the 'natural_log_exp_and_others' activation
    # table set; left to itself the table-load pass picks per-function sets
    # and the Scalar engine reloads tables on every Exp<->Ln alternation
    # (~1.3us each, ~46us total here). Restrict selection to the shared set
    # (other sets are emptied, keeping dict order so act_func_set_id indices
    # stay aligned with act_info.json).
    from concourse.hw_specs import get_activation_tables as _real_gat

    def _patched_gat(arch):
        tabs = _real_gat(arch)
        return {
            k: (v if k == "natural_log_exp_and_others" else set())
            for k, v in tabs.items()
        }

    bacc.get_activation_tables = _patched_gat

    nc = bacc.Bacc(
        "TRN2",
        target_bir_lowering=False,
        debug=False,
        num_devices=NCORES,
    )

    # ---- DRAM parameters -------------------------------------------------
    embT_d = nc.declare_dram_parameter("embT", [D, NC_NODES], dt.float32, isOutput=False)
    cwT_d = nc.declare_dram_parameter("cwT", [D, 1024], dt.float32, isOutput=False)
    par_d = nc.declare_dram_parameter("par", [128, 12], dt.float32, isOutput=False)
    rtpar_d = nc.declare_dram_parameter("rtpar", [NC_EV, 4], dt.float32, isOutput=False)
    tdb_d = nc.declare_dram_parameter("tdb", [NC_EV, 2 * S], dt.float32, isOutput=False)

    lam_d = nc.declare_dram_parameter("lam", [1024, NC_NODES], dt.float32, isOutput=True)
    rtp_d = nc.declare_dram_parameter("rtp", [NC_EV, 1], dt.float32, isOutput=True)

    F32R = dt.float32r

    with tile.TileContext(nc) as tc:
        with tc.tile_pool(name="const", bufs=1) as cpool, \
             tc.tile_pool(name="stage", bufs=3) as spool:
            sb_embTr = cpool.tile([D, NC_NODES], F32R)
            sb_cwTr = cpool.tile([D, 1024], F32R)
            sb_par = cpool.tile([128, 12], dt.float32)
            sb_rtpar = cpool.tile([NC_EV, 4], dt.float32)
            sb_tdb = cpool.tile([NC_EV, 2 * S], dt.float32)

            # small params issue from the Activation HWDGE queue, the big
            # embT stream from the SP queue - the issues overlap.
            nc.scalar.dma_start(sb_tdb[:], tdb_d[:])
            nc.scalar.dma_start(sb_rtpar[:], rtpar_d[:])
            nc.scalar.dma_start(sb_par[:], par_d[:])
            st = spool.tile([D, 1024], dt.float32, tag="st")
            nc.sync.dma_start(st[:], cwT_d[:])
            nc.vector.tensor_copy(sb_cwTr[:], st[:])
            # embT load + fp32r rounding, pipelined through small staging
            # tiles so the first lambda matmuls start while the rest streams.
            n_ld = 8
            ld = (NC_NODES + n_ld - 1) // n_ld  # 782
            for i in range(n_ld):
                lo = i * ld
                hi = min(NC_NODES, (i + 1) * ld)
                st = spool.tile([D, 1024], dt.float32, tag="st")
                nc.sync.dma_start(st[:, 0 : hi - lo], embT_d[:, lo:hi])
                nc.vector.tensor_copy(sb_embTr[:, lo:hi], st[:, 0 : hi - lo])

            # ========== return_time_pred (64 events, no PSUM) ==========
            with tc.tile_pool(name="rt_sb", bufs=1) as rpool:
                ev = NC_EV
                E = rpool.tile([ev, S], dt.float32)
                Ften = rpool.tile([ev, S], dt.float32)
                I = rpool.tile([ev, S], dt.float32)
                cum = rpool.tile([ev, S], dt.float32)
                dens = rpool.tile([ev, S], dt.float32)
                rtp_sb = rpool.tile([ev, 1], dt.float32)

                sgrid = sb_tdb[:, 0:S]
                wtd = sb_tdb[:, S : 2 * S]

                # E = exp(-w_t/1e4 * s)
                nc.scalar.activation(E[:], sgrid, AF.Exp, scale=sb_rtpar[:, 2:3])
                # F = exp(rt_scale*E + rt_bias)
                nc.scalar.activation(
                    Ften[:], E[:], AF.Exp,
                    scale=sb_rtpar[:, 1:2], bias=sb_rtpar[:, 0:1],
                )
                # I = psi * ln(F + 1)
                nc.scalar.activation(I[:], Ften[:], AF.Ln, bias=1.0)
                nc.vector.tensor_scalar_mul(I[:], I[:], sb_rtpar[:, 3:4])

                def _emit_rtp_tail():
                    # inclusive cumsum along s (fp32 state)
                    nc.vector.tensor_tensor_scan(
                        cum[:], I[:], I[:], 0.0,
                        mybir.AluOpType.add, mybir.AluOpType.bypass,
                    )
                    # X = exp(-TIMESTEP*cum); density = I * X (reuse cum)
                    nc.scalar.activation(cum[:], cum[:], AF.Exp, scale=-TIMESTEP)
                    nc.vector.tensor_tensor(
                        dens[:], cum[:], I[:], mybir.AluOpType.mult
                    )
                    # ts = density * (trapz_w * td); rtp = sum_s ts
                    nc.vector.tensor_tensor(
                        dens[:], dens[:], wtd, mybir.AluOpType.mult
                    )
                    nc.vector.tensor_reduce(
                        rtp_sb[:], dens[:], mybir.AxisListType.X,
                        mybir.AluOpType.add,
                    )
                    nc.sync.dma_start(rtp_d[:], rtp_sb[:])

                # ========== lambda_src / lambda_dst main loop ==========
                # Row-major over the 8 (type, event-group) rows; each row
                # covers all 6250 nodes in one eb tile. Exp drains PSUM at
                # 2048 granularity; Ln runs in-place per half-row (3125) so
                # the output DMA of each half starts early and the final
                # DMA tail stays short.
                CH = 2048
                HALF = 3125
                with tc.tile_pool(name="lam_ps", bufs=2, space="PSUM") as lps, \
                     tc.tile_pool(name="lam_e", bufs=2) as epool:
                    for tg in range(8):
                        g = tg % 4
                        eb = epool.tile([128, NC_NODES], dt.float32, tag="eb")
                        for off in range(0, NC_NODES, CH):
                            F = min(CH, NC_NODES - off)
                            ps = lps.tile([128, CH], dt.float32, tag="lps")
                            for s1 in range(0, F, 512):
                                ss = min(512, F - s1)
                                nc.tensor.matmul(
                                    ps[:, s1 : s1 + ss],
                                    sb_cwTr[:, tg * 128 : (tg + 1) * 128],
                                    sb_embTr[:, off + s1 : off + s1 + ss],
                                    start=True,
                                    stop=True,
                                )
                            nc.scalar.activation(
                                eb[:, off : off + F], ps[:, 0:F], AF.Exp,
                                bias=sb_par[:, tg : tg + 1],
                            )
                        for off in (0, HALF):
                            F = min(HALF, NC_NODES - off)
                            sl = eb[:, off : off + F]
                            nc.scalar.activation(sl, sl, AF.Ln, bias=1.0)
                            nc.vector.tensor_scalar_mul(
                                sl, sl, sb_par[:, 8 + g : 9 + g]
                            )
                            nc.sync.dma_start(
                                lam_d[tg * 128 : (tg + 1) * 128, off : off + F],
                                sl,
                            )
                        if tg == 0:
                            # rtp integral: its DVE scan ran during row 0's
                            # ACT work; the single Exp slots in here without
                            # stalling the in-order Scalar queue.
                            _emit_rtp_tail()

    nc.compile()
    return nc


def _get_program():
    if "nc" not in _PROGRAM_CACHE:
        _PROGRAM_CACHE["nc"] = _build_program()
    return _PROGRAM_CACHE["nc"]


def _host_prep(all_embeddings, assoc, src, pos_dst, last_update, cur_time, et,
               W, b, psi, alpha, w_t):
    """Per-event scalar prep (O(B*D)) + shard layouts. float64 intermediate
    for the tiny scalar math, cast to float32 for upload."""
    emb = np.asarray(all_embeddings, dtype=np.float32)
    assoc = np.asarray(assoc).astype(np.int64)
    src = np.asarray(src).astype(np.int64)
    pos_dst = np.asarray(pos_dst).astype(np.int64)
    lu = np.asarray(last_update, dtype=np.float64)
    ct = np.asarray(cur_time, dtype=np.float64)
    e = np.asarray(et).astype(np.int64)
    e = (e > 0).astype(np.int64)
    W = np.asarray(W, dtype=np.float64)
    bb = np.asarray(b, dtype=np.float64)
    psi = np.asarray(psi, dtype=np.float64)
    alpha = np.asarray(alpha, dtype=np.float64)
    w_t = np.asarray(w_t, dtype=np.float64)

    Wu = W[:, :D]  # (2, D)
    Wv = W[:, D:]

    idx_s = assoc[src]
    idx_d = assoc[pos_dst]
    zs = emb[idx_s].astype(np.float64)  # (B, D)
    zd = emb[idx_d].astype(np.float64)
    td_s = ct - lu[idx_s]
    td_d = ct - lu[idx_d]

    invpsi = 1.0 / (psi + 1e-7)
    ip = invpsi[e]  # (B,)
    psi_e = psi[e]
    alpha_e = alpha[e]
    wt_e = w_t[e]
    b_e = bb[e]

    a_s = np.einsum("bk,bk->b", zs, Wu[e])
    a_d = np.einsum("bk,bk->b", zd, Wv[e])
    wb_s = ip * (a_s + b_e + alpha_e * np.exp(-wt_e * td_s / TRAIN_TD_MAX))
    wb_d = ip * (a_d + b_e + alpha_e * np.exp(-wt_e * td_d / TRAIN_TD_MAX))

    cw_s = ip[:, None] * Wv[e]  # (B, D)   lambda_src node side goes via Wv
    cw_d = ip[:, None] * Wu[e]  # lambda_dst node side via Wu

    # cwT (D, 1024): col block tg = t*4+g holds events g*128..(g+1)*128 of type t
    cwT = np.zeros((D, 1024), dtype=np.float32)
    par = np.zeros((128, 12), dtype=np.float32)
    for g in range(4):
        sl = slice(g * 128, (g + 1) * 128)
        cwT[:, (0 * 4 + g) * 128 : (0 * 4 + g + 1) * 128] = cw_s[sl].T
        cwT[:, (1 * 4 + g) * 128 : (1 * 4 + g + 1) * 128] = cw_d[sl].T
        par[:, 0 * 4 + g] = wb_s[sl]
        par[:, 1 * 4 + g] = wb_d[sl]
        par[:, 8 + g] = psi_e[sl]

    # rtp per-event scalars
    base = a_s + np.einsum("bk,bk->b", zd, Wv[e]) + b_e
    rt_bias = ip * base
    rt_scale = ip * alpha_e
    nws = -wt_e * (TIMESTEP / TRAIN_TD_MAX)  # exp(nws * s), s integer
    rtpar_full = np.stack(
        [rt_bias, rt_scale, nws, psi_e], axis=1
    ).astype(np.float32)  # (B, 4)

    # tdb2 (64, 2S): [s grid | trapezoid_weight * td]
    s_vals = np.arange(S, dtype=np.float64)
    w = np.full(S, TIMESTEP)
    w[-1] = 0.5 * TIMESTEP
    wtd = (w * (TIMESTEP * s_vals)).astype(np.float32)
    tdb2 = np.zeros((NC_EV, 2 * S), dtype=np.float32)
    tdb2[:, 0:S] = s_vals.astype(np.float32)[None, :]
    tdb2[:, S:] = wtd[None, :]

    # per-core embT slices (pre-transposed layout)
    embT_slices = []
    for c in range(NCORES):
        sl = emb[c * NC_NODES : (c + 1) * NC_NODES, :]
        embT_slices.append(np.ascontiguousarray(sl.T))

    rtpar_slices = [
        np.ascontiguousarray(rtpar_full[c * NC_EV : (c + 1) * NC_EV])
        for c in range(NCORES)
    ]

    return cwT, par, tdb2, embT_slices, rtpar_slices


def kernel(all_embeddings, assoc, src, pos_dst, neg_dst, last_update,
           cur_time, et, W, b, psi, alpha, w_t):
    from concourse.bass_utils import run_bass_kernel_spmd

    cwT, par, tdb2, embT_slices, rtpar_slices = _host_prep(
        all_embeddings, assoc, src, pos_dst, last_update, cur_time, et,
        W, b, psi, alpha, w_t,
    )

    nc = _get_program()

    in_maps = []
    for c in range(NCORES):
        in_maps.append({
            "embT": embT_slices[c],
            "cwT": cwT,
            "par": par,
            "rtpar": rtpar_slices[c],
            "tdb": tdb2,
        })

    res = run_bass_kernel_spmd(nc, in_maps, core_ids=list(range(NCORES))).results

    lam_parts = [res[c]["lam"] for c in range(NCORES)]  # (1024, 6250) each
    lambda_src = np.concatenate([p[:512] for p in lam_parts], axis=1)
    lambda_dst = np.concatenate([p[512:] for p in lam_parts], axis=1)
    rtp = np.concatenate([res[c]["rtp"].reshape(NC_EV) for c in range(NCORES)])
    return lambda_src, lambda_dst, rtp


# revision 24
# speedup vs baseline: 1.2279x; 1.0264x over previous
"""Trainium2 Bass kernel for nn_Decoder (Hawkes intensity decoder).

Contract: kernel(**inputs) takes FULL unsharded inputs (as produced by the
reference's setup_inputs) and returns the full (lambda_src, lambda_dst,
return_time_pred) tuple.

Sharding (8 NeuronCores):
  - lambda_src/lambda_dst (B=512 x N=50000): node-sharded. Core c computes
    ALL 512 events against its 6250-node slice of all_embeddings. This cuts
    the per-core input DMA 8x vs batch-sharding (only the node slice is
    read) while output DMA (the roofline: 2 x 102.4MB fp32) is identical.
  - return_time_pred (B=512): batch-sharded, 64 events per core.

Per-core math. hawkes_intensity separates per (event b, node n):
    g[b,n] = z_ev[b].Wa[et_b] + emb[n].Wb[et_b] + bias[et_b]
             + alpha[et_b]*exp(-w_t[et_b]*td_b/100)
so with x = g/(psi+1e-7),
    lambda[b,n] = psi_e*(logaddexp(0,-x)+x) = psi_e*ln(1+exp(x)).
The node-independent part folds into a per-event scalar w_b; the node part
is a K=32 matmul row: x[b,n] = CW[b,:].emb[n,:] + w_b with
CW[b,:] = Wb[et_b]/(psi_e+1e-7). On device:
    PE   : PSUM[128ev, n] = cwT^T @ embT     (fp32r, K=32)
    ACT  : e = Exp(PSUM + w_b)  (per-partition bias)
    ACT  : l = Ln(e + 1)
    DVE  : out = l * psi_b      (per-partition scalar)
    DMA  : out -> lam[...]
|x| <~ 25 for any plausible input here, so Ln(1+Exp(x)) is overflow-safe in
fp32 and equals the reference's stable logaddexp form.

return_time_pred per core (64 events on partitions, s=0..1000 on the free
dim; tdb2 carries [s grid | trapezoid-weight*td]):
    E = Exp(s * (-w_t_b*1e-4)); F = Exp(rt_scale_b*E + rt_bias_b)
    I = psi_b*Ln(F+1)                       # intensity (b, s)
    cum = tensor_tensor_scan(I, add)        # inclusive cumsum along s (DVE)
    X = Exp(-0.01*cum); density = I*X
    rtp = reduce_sum(density * wtd, axis=s) # wtd = trapz weight * td
"""

import numpy as np

N_NODES = 50000
B = 512
D = 32
NCORES = 8
NC_NODES = N_NODES // NCORES  # 6250
NC_EV = B // NCORES  # 64
S = 1001
TRAIN_TD_MAX = 100.0
TIMESTEP = 0.01

_PROGRAM_CACHE = {}


def _build_program():
    """Build + compile the SPMD Bass program (identical on all 8 cores)."""
    import concourse.bass as bass
    import concourse.mybir as mybir
    from concourse import bacc, tile

    dt = mybir.dt
    AF = mybir.ActivationFunctionType

    # Both Exp and Ln live in the 'natural_log_exp_and_others' activation
    # table set; left to itself the table-load pass picks per-function sets
    # and the Scalar engine reloads tables on every Exp<->Ln alternation
    # (~1.3us each, ~46us total here). Restrict selection to the shared set
    # (other sets are emptied, keeping dict order so act_func_set_id indices
    # stay aligned with act_info.json).
    from concourse.hw_specs import get_activation_tables as _real_gat

    def _patched_gat(arch):
        tabs = _real_gat(arch)
        return {
            k: (v if k == "natural_log_exp_and_others" else set())
            for k, v in tabs.items()
        }

    bacc.get_activation_tables = _patched_gat

    nc = bacc.Bacc(
        "TRN2",
        target_bir_lowering=False,
        debug=False,
        num_devices=NCORES,
    )

    # ---- DRAM parameters -------------------------------------------------
    embT_d = nc.declare_dram_parameter("embT", [D, NC_NODES], dt.float32, isOutput=False)
    cwT_d = nc.declare_dram_parameter("cwT", [D, 1024], dt.float32, isOutput=False)
    par_d = nc.declare_dram_parameter("par", [128, 12], dt.float32, isOutput=False)
    rtpar_d = nc.declare_dram_parameter("rtpar", [NC_EV, 4], dt.float32, isOutput=False)
    tdb_d = nc.declare_dram_parameter("tdb", [NC_EV, 2 * S], dt.float32, isOutput=False)

    lam_d = nc.declare_dram_parameter("lam", [1024, NC_NODES], dt.float32, isOutput=True)
    rtp_d = nc.declare_dram_parameter("rtp", [NC_EV, 1], dt.float32, isOutput=True)

    F32R = dt.float32r

    with tile.TileContext(nc) as tc:
        with tc.tile_pool(name="const", bufs=1) as cpool, \
             tc.tile_pool(name="stage", bufs=3) as spool:
            sb_embTr = cpool.tile([D, NC_NODES], F32R)
            sb_cwTr = cpool.tile([D, 1024], F32R)
            sb_par = cpool.tile([128, 12], dt.float32)
            sb_rtpar = cpool.tile([NC_EV, 4], dt.float32)
            sb_tdb = cpool.tile([NC_EV, 2 * S], dt.float32)

            # The Activation HWDGE queue finishes its preamble ~3us before
            # SP's, so the first-matmul dependencies (embT chunk 0, cwT) and
            # the small params issue there; the remaining embT chunks stream
            # from the SP queue in parallel.
            n_ld = 8
            ld = (NC_NODES + n_ld - 1) // n_ld  # 782
            st0 = spool.tile([D, 1024], dt.float32, tag="st")
            nc.scalar.dma_start(st0[:, 0:ld], embT_d[:, 0:ld])
            stc = spool.tile([D, 1024], dt.float32, tag="st")
            nc.scalar.dma_start(stc[:], cwT_d[:])
            nc.scalar.dma_start(sb_par[:], par_d[:])
            nc.scalar.dma_start(sb_tdb[:], tdb_d[:])
            nc.scalar.dma_start(sb_rtpar[:], rtpar_d[:])
            nc.vector.tensor_copy(sb_embTr[:, 0:ld], st0[:, 0:ld])
            nc.vector.tensor_copy(sb_cwTr[:], stc[:])
            for i in range(1, n_ld):
                lo = i * ld
                hi = min(NC_NODES, (i + 1) * ld)
                st = spool.tile([D, 1024], dt.float32, tag="st")
                nc.sync.dma_start(st[:, 0 : hi - lo], embT_d[:, lo:hi])
                nc.vector.tensor_copy(sb_embTr[:, lo:hi], st[:, 0 : hi - lo])

            # ========== return_time_pred (64 events, no PSUM) ==========
            with tc.tile_pool(name="rt_sb", bufs=1) as rpool:
                ev = NC_EV
                E = rpool.tile([ev, S], dt.float32)
                Ften = rpool.tile([ev, S], dt.float32)
                I = rpool.tile([ev, S], dt.float32)
                cum = rpool.tile([ev, S], dt.float32)
                dens = rpool.tile([ev, S], dt.float32)
                rtp_sb = rpool.tile([ev, 1], dt.float32)

                sgrid = sb_tdb[:, 0:S]
                wtd = sb_tdb[:, S : 2 * S]

                # E = exp(-w_t/1e4 * s)
                nc.scalar.activation(E[:], sgrid, AF.Exp, scale=sb_rtpar[:, 2:3])
                # F = exp(rt_scale*E + rt_bias)
                nc.scalar.activation(
                    Ften[:], E[:], AF.Exp,
                    scale=sb_rtpar[:, 1:2], bias=sb_rtpar[:, 0:1],
                )
                # I = psi * ln(F + 1)
                nc.scalar.activation(I[:], Ften[:], AF.Ln, bias=1.0)
                nc.vector.tensor_scalar_mul(I[:], I[:], sb_rtpar[:, 3:4])

                def _emit_rtp_tail():
                    # inclusive cumsum along s (fp32 state)
                    nc.vector.tensor_tensor_scan(
                        cum[:], I[:], I[:], 0.0,
                        mybir.AluOpType.add, mybir.AluOpType.bypass,
                    )
                    # X = exp(-TIMESTEP*cum); density = I * X (reuse cum)
                    nc.scalar.activation(cum[:], cum[:], AF.Exp, scale=-TIMESTEP)
                    nc.vector.tensor_tensor(
                        dens[:], cum[:], I[:], mybir.AluOpType.mult
                    )
                    # ts = density * (trapz_w * td); rtp = sum_s ts
                    nc.vector.tensor_tensor(
                        dens[:], dens[:], wtd, mybir.AluOpType.mult
                    )
                    nc.vector.tensor_reduce(
                        rtp_sb[:], dens[:], mybir.AxisListType.X,
                        mybir.AluOpType.add,
                    )
                    nc.sync.dma_start(rtp_d[:], rtp_sb[:])

                # ========== lambda_src / lambda_dst main loop ==========
                # Row-major over the 8 (type, event-group) rows; each row
                # covers all 6250 nodes in one eb tile. Exp drains PSUM at
                # 2048 granularity; Ln runs in-place per half-row (3125) so
                # the output DMA of each half starts early and the final
                # DMA tail stays short.
                CH = 2048
                HALF = 3125
                with tc.tile_pool(name="lam_ps", bufs=2, space="PSUM") as lps, \
                     tc.tile_pool(name="lam_e", bufs=3) as epool:
                    for tg in range(8):
                        g = tg % 4
                        eb = epool.tile([128, NC_NODES], dt.float32, tag="eb")
                        for off in range(0, NC_NODES, CH):
                            F = min(CH, NC_NODES - off)
                            ps = lps.tile([128, CH], dt.float32, tag="lps")
                            for s1 in range(0, F, 512):
                                ss = min(512, F - s1)
                                nc.tensor.matmul(
                                    ps[:, s1 : s1 + ss],
                                    sb_cwTr[:, tg * 128 : (tg + 1) * 128],
                                    sb_embTr[:, off + s1 : off + s1 + ss],
                                    start=True,
                                    stop=True,
                                )
                            nc.scalar.activation(
                                eb[:, off : off + F], ps[:, 0:F], AF.Exp,
                                bias=sb_par[:, tg : tg + 1],
                            )
                        if tg < 7:
                            pieces = [(0, HALF), (HALF, NC_NODES - HALF)]
                        else:
                            # taper the final row so the last Ln->mult->DMA
                            # chain (the kernel tail) is short
                            pieces = [(0, HALF), (HALF, 1563), (4688, 1041),
                                      (5729, 521)]
                        for off, F in pieces:
                            sl = eb[:, off : off + F]
                            nc.scalar.activation(sl, sl, AF.Ln, bias=1.0)
                            nc.vector.tensor_scalar_mul(
                                sl, sl, sb_par[:, 8 + g : 9 + g]
                            )
                            nc.sync.dma_start(
                                lam_d[tg * 128 : (tg + 1) * 128, off : off + F],
                                sl,
                            )
                        if tg == 0:
                            # rtp integral: its DVE scan ran during row 0's
                            # ACT work; the single Exp slots in here without
                            # stalling the in-order Scalar queue.
                            _emit_rtp_tail()

    nc.compile()
    return nc


def _get_program():
    if "nc" not in _PROGRAM_CACHE:
        _PROGRAM_CACHE["nc"] = _build_program()
    return _PROGRAM_CACHE["nc"]


def _host_prep(all_embeddings, assoc, src, pos_dst, last_update, cur_time, et,
               W, b, psi, alpha, w_t):
    """Per-event scalar prep (O(B*D)) + shard layouts. float64 intermediate
    for the tiny scalar math, cast to float32 for upload."""
    emb = np.asarray(all_embeddings, dtype=np.float32)
    assoc = np.asarray(assoc).astype(np.int64)
    src = np.asarray(src).astype(np.int64)
    pos_dst = np.asarray(pos_dst).astype(np.int64)
    lu = np.asarray(last_update, dtype=np.float64)
    ct = np.asarray(cur_time, dtype=np.float64)
    e = np.asarray(et).astype(np.int64)
    e = (e > 0).astype(np.int64)
    W = np.asarray(W, dtype=np.float64)
    bb = np.asarray(b, dtype=np.float64)
    psi = np.asarray(psi, dtype=np.float64)
    alpha = np.asarray(alpha, dtype=np.float64)
    w_t = np.asarray(w_t, dtype=np.float64)

    Wu = W[:, :D]  # (2, D)
    Wv = W[:, D:]

    idx_s = assoc[src]
    idx_d = assoc[pos_dst]
    zs = emb[idx_s].astype(np.float64)  # (B, D)
    zd = emb[idx_d].astype(np.float64)
    td_s = ct - lu[idx_s]
    td_d = ct - lu[idx_d]

    invpsi = 1.0 / (psi + 1e-7)
    ip = invpsi[e]  # (B,)
    psi_e = psi[e]
    alpha_e = alpha[e]
    wt_e = w_t[e]
    b_e = bb[e]

    a_s = np.einsum("bk,bk->b", zs, Wu[e])
    a_d = np.einsum("bk,bk->b", zd, Wv[e])
    wb_s = ip * (a_s + b_e + alpha_e * np.exp(-wt_e * td_s / TRAIN_TD_MAX))
    wb_d = ip * (a_d + b_e + alpha_e * np.exp(-wt_e * td_d / TRAIN_TD_MAX))

    cw_s = ip[:, None] * Wv[e]  # (B, D)   lambda_src node side goes via Wv
    cw_d = ip[:, None] * Wu[e]  # lambda_dst node side via Wu

    # cwT (D, 1024): col block tg = t*4+g holds events g*128..(g+1)*128 of type t
    cwT = np.zeros((D, 1024), dtype=np.float32)
    par = np.zeros((128, 12), dtype=np.float32)
    for g in range(4):
        sl = slice(g * 128, (g + 1) * 128)
        cwT[:, (0 * 4 + g) * 128 : (0 * 4 + g + 1) * 128] = cw_s[sl].T
        cwT[:, (1 * 4 + g) * 128 : (1 * 4 + g + 1) * 128] = cw_d[sl].T
        par[:, 0 * 4 + g] = wb_s[sl]
        par[:, 1 * 4 + g] = wb_d[sl]
        par[:, 8 + g] = psi_e[sl]

    # rtp per-event scalars
    base = a_s + np.einsum("bk,bk->b", zd, Wv[e]) + b_e
    rt_bias = ip * base
    rt_scale = ip * alpha_e
    nws = -wt_e * (TIMESTEP / TRAIN_TD_MAX)  # exp(nws * s), s integer
    rtpar_full = np.stack(
        [rt_bias, rt_scale, nws, psi_e], axis=1
    ).astype(np.float32)  # (B, 4)

    # tdb2 (64, 2S): [s grid | trapezoid_weight * td]
    s_vals = np.arange(S, dtype=np.float64)
    w = np.full(S, TIMESTEP)
    w[-1] = 0.5 * TIMESTEP
    wtd = (w * (TIMESTEP * s_vals)).astype(np.float32)
    tdb2 = np.zeros((NC_EV, 2 * S), dtype=np.float32)
    tdb2[:, 0:S] = s_vals.astype(np.float32)[None, :]
    tdb2[:, S:] = wtd[None, :]

    # per-core embT slices (pre-transposed layout)
    embT_slices = []
    for c in range(NCORES):
        sl = emb[c * NC_NODES : (c + 1) * NC_NODES, :]
        embT_slices.append(np.ascontiguousarray(sl.T))

    rtpar_slices = [
        np.ascontiguousarray(rtpar_full[c * NC_EV : (c + 1) * NC_EV])
        for c in range(NCORES)
    ]

    return cwT, par, tdb2, embT_slices, rtpar_slices


def kernel(all_embeddings, assoc, src, pos_dst, neg_dst, last_update,
           cur_time, et, W, b, psi, alpha, w_t):
    from concourse.bass_utils import run_bass_kernel_spmd

    cwT, par, tdb2, embT_slices, rtpar_slices = _host_prep(
        all_embeddings, assoc, src, pos_dst, last_update, cur_time, et,
        W, b, psi, alpha, w_t,
    )

    nc = _get_program()

    in_maps = []
    for c in range(NCORES):
        in_maps.append({
            "embT": embT_slices[c],
            "cwT": cwT,
            "par": par,
            "rtpar": rtpar_slices[c],
            "tdb": tdb2,
        })

    res = run_bass_kernel_spmd(nc, in_maps, core_ids=list(range(NCORES))).results

    lam_parts = [res[c]["lam"] for c in range(NCORES)]  # (1024, 6250) each
    lambda_src = np.concatenate([p[:512] for p in lam_parts], axis=1)
    lambda_dst = np.concatenate([p[512:] for p in lam_parts], axis=1)
    rtp = np.concatenate([res[c]["rtp"].reshape(NC_EV) for c in range(NCORES)])
    return lambda_src, lambda_dst, rtp
